# revision 14
# baseline (speedup 1.0000x reference)
"""Trainium2 Bass kernel for batched self-attention with input projections.

Problem: B=8, N=2048, D=131
    Q = q @ Wq.T + bq;  K = k @ Wk.T + bk;  V = v @ Wv.T + bv
    out = softmax(Q K^T / sqrt(131)) V

One batch element per NeuronCore (8 cores, no communication).

Host prep (layout/algebra only):
  - Tokens augmented with a ones-row: X = [x^T; 1] in [132, 2048] so biases
    fold into the projection matmuls.
  - Scores: Q K^T = Xq (Wq'^T Wk'/sqrt(D)) Xk^T = Xq G Xk^T, G [132,132].
    SVD-truncate G to rank 128 (exact rank 131; error ~2e-5) so the big S
    matmul is a single K=128 contraction:  S = (Xq Aq)(Xk Ak)^T.
  - Value path: W2 [132,132] maps X -> [V | 1] (bias row + denominator
    ones-column).  SVD-truncate W2 = L R^T to rank 128 so the O-matmul
    contracts into a 128-wide latent:  O' = (P Xv L) R^T, with O'[:,131]
    the softmax denominator.  Measured end-to-end rel err ~3.8e-3 in bf16.
  - Everything bf16 (PE runs bf16 at 4x fp32); fp32 PSUM accumulation.
    |S| < 3 so softmax without max-subtraction is safe.

Per core:
  QT[e',n] = Aq^T Xq, KT[e',n] = Ak^T Xk      (2 d-chunks: 0:128, 128:132)
  VL[n,l]  = Xv^T L                            (16 j-blocks of [128,128])
  for i-half h (1024 cols), j-block (16):
      ST = KT_j^T QT[:,h]  -> exp on ACT -> E (bf16, [128,1024])
      Ohat^T[l, h] += VL_j^T-matmul with E     (PSUM accumulate over j)
  O'[i,132] = Ohat_i R^T;  out = O'[:,0:131] / O'[:,131]

All SBUF projection tensors are chunked [128,512] tiles so the attention
stream starts as soon as its first chunks are projected; a burst of junk
matmuls during the input DMA warms the PE clock gate (HAM); a post-finalize
pass drops Ldweights instructions that reload the identical weights.
"""

import numpy as np
import ml_dtypes

P = 128          # partitions / PE width
N = 2048         # tokens per core
D = 131          # embed dim
DP = 132         # embed dim + ones row
DLO = DP - P     # tail contraction rows (4)
R = 128          # truncated rank (QK interaction and V latent)
EV = 132         # final output cols (131 + denominator)
NB = N // P      # 16 token blocks
HW = 1024        # i-half width
NH = N // HW     # 2 halves
NCORES = 8

QOFF, KOFF, VOFF = 0, N, 2 * N          # column offsets in packed xall
AQOFF, AKOFF, LOFF = 0, R, 2 * R        # column offsets in packed weights

_BF16 = ml_dtypes.bfloat16


def build_nc():
    """Build the single-core Bass graph (same NEFF runs SPMD on all 8 cores)."""
    from contextlib import ExitStack

    import concourse.bacc as bacc
    import concourse.mybir as mybir
    import concourse.tile as tile
    from concourse.bass import ts

    bf = mybir.dt.bfloat16
    f32 = mybir.dt.float32
    EXP = mybir.ActivationFunctionType.Exp
    COPY = mybir.ActivationFunctionType.Copy

    nc = bacc.Bacc()
    xall = nc.declare_dram_parameter("xall", [DP, 3 * N], bf, isOutput=False)
    wpack = nc.declare_dram_parameter("wpack", [DP, 3 * R], bf, isOutput=False)
    rmat = nc.declare_dram_parameter("rmat", [R, EV], bf, isOutput=False)
    out = nc.declare_dram_parameter("out", [N, D], f32, isOutput=True)

    with tile.TileContext(nc) as tc, ExitStack() as ctx:
        const = ctx.enter_context(tc.tile_pool(name="const", bufs=1))
        xin = ctx.enter_context(tc.tile_pool(name="xin", bufs=1))
        proj = ctx.enter_context(tc.tile_pool(name="proj", bufs=1))
        epool = ctx.enter_context(tc.tile_pool(name="epool", bufs=NB + 2))
        ohs = ctx.enter_context(tc.tile_pool(name="ohs", bufs=1))
        outp = ctx.enter_context(tc.tile_pool(name="outp", bufs=2))
        warm = ctx.enter_context(tc.tile_pool(name="warm", bufs=1))
        # PSUM budget (8 banks): proj/final 2 x [128,512] = 2, scores
        # 2 x [128,1024] = 4, Ohat accumulator 1 x [128,1024] = 2.
        psp = ctx.enter_context(tc.tile_pool(name="psp", bufs=2, space="PSUM"))
        psst = ctx.enter_context(tc.tile_pool(name="psst", bufs=2, space="PSUM"))
        psoh = ctx.enter_context(tc.tile_pool(name="psoh", bufs=1, space="PSUM"))

        # ---- DMA loads.  The big inputs are split into 4 partition-range
        # chunks each: 32 descriptors per dma_start (fast issue) and the
        # chunks spread over the 8 HWDGE queues so transfers run parallel.
        wp_hi = const.tile([P, 3 * R], bf)
        nc.sync.dma_start(out=wp_hi, in_=wpack[0:P, :])
        wp_lo = const.tile([DLO, 3 * R], bf)
        nc.sync.dma_start(out=wp_lo, in_=wpack[P:DP, :])
        xall_hi = xin.tile([P, 3 * N], bf)
        xall_lo = xin.tile([DLO, 3 * N], bf)
        nc.sync.dma_start(out=xall_lo, in_=xall[P:DP, :])
        for off in (QOFF, KOFF, VOFF):
            for s in range(0, P, 32):
                nc.sync.dma_start(
                    out=xall_hi[s:s + 32, off:off + N],
                    in_=xall[s:s + 32, off:off + N],
                )
        rmat_s = const.tile([R, EV], bf)
        nc.sync.dma_start(out=rmat_s, in_=rmat[:, :])

        # ---- PE warm-up during the DMA window: dense junk matmuls flip the
        # HAM clock gate (~3.4us of sustained activity) so the projections
        # run at 2.4GHz.  No data deps -> the scheduler front-loads them.
        wsrc = warm.tile([P, 512], bf)
        nc.vector.memset(wsrc, 0)
        for w in range(18):
            pw = psst.tile([P, HW], f32, tag="pst", name="pw")
            nc.tensor.matmul(pw[:, 0:512], wsrc[:, 0:P], wsrc, start=True, stop=True)

        # ---- projections, chunked so attention can start early.
        # Order: the h=0 attention stream needs QT chunks 0-1 and all KT
        # chunks, so project those first; QT 2-3 (h=1) and VL (Ohat) follow.
        qts = [proj.tile([P, 512], bf, tag=f"qt{c}", name=f"qt{c}") for c in range(4)]
        kts = [proj.tile([P, 512], bf, tag=f"kt{c}", name=f"kt{c}") for c in range(4)]
        vls = [proj.tile([P, 512], bf, tag=f"vl{c}", name=f"vl{c}") for c in range(4)]

        def qk_chunk(dst, woff, xoff, c):
            pp = psp.tile([P, 512], f32, tag="pp", name="pp")
            nc.tensor.matmul(
                pp,
                wp_hi[:, woff:woff + R],
                xall_hi[:, xoff + c * 512: xoff + (c + 1) * 512],
                start=True,
                stop=False,
            )
            nc.tensor.matmul(
                pp,
                wp_lo[:, woff:woff + R],
                xall_lo[:, xoff + c * 512: xoff + (c + 1) * 512],
                start=False,
                stop=True,
            )
            nc.vector.tensor_copy(dst, pp)

        for c in range(2):
            qk_chunk(qts[c], AQOFF, QOFF, c)
        for c in range(4):
            qk_chunk(kts[c], AKOFF, KOFF, c)
        for c in range(2, 4):
            qk_chunk(qts[c], AQOFF, QOFF, c)
        for g in range(4):
            pv = psp.tile([P, 512], f32, tag="pp", name="pv")
            for t in range(4):
                j = 4 * g + t
                nc.tensor.matmul(
                    pv[:, ts(t, P)],
                    xall_hi[:, VOFF + j * P: VOFF + (j + 1) * P],
                    wp_hi[:, LOFF:LOFF + R],
                    start=True,
                    stop=False,
                )
                nc.tensor.matmul(
                    pv[:, ts(t, P)],
                    xall_lo[:, VOFF + j * P: VOFF + (j + 1) * P],
                    wp_lo[:, LOFF:LOFF + R],
                    start=False,
                    stop=True,
                )
            nc.vector.tensor_copy(vls[g], pv)

        # ---- attention + per-half finalization ----
        for h in range(NH):
            es = []
            for j in range(NB):
                pst = psst.tile([P, HW], f32, tag="pst", name="pst")
                for c in range(2):
                    nc.tensor.matmul(
                        pst[:, ts(c, 512)],
                        kts[j // 4][:, ts(j % 4, P)],
                        qts[2 * h + c],
                        start=True,
                        stop=True,
                    )
                ej = epool.tile([P, HW], bf, tag="e", name="ej")
                nc.scalar.activation(ej, pst, EXP)
                es.append(ej)
            poh = psoh.tile([P, HW], f32, tag="poh", name="poh")
            for j in range(NB):
                for c in range(2):
                    nc.tensor.matmul(
                        poh[:, ts(c, 512)],
                        vls[j // 4][:, ts(j % 4, P)],
                        es[j][:, ts(c, 512)],
                        start=(j == 0),
                        stop=(j == NB - 1),
                    )
            ohat = ohs.tile([P, HW], bf, tag=f"oh{h}", name=f"oh{h}")
            if h == 0:
                nc.vector.tensor_copy(ohat, poh)
            else:
                # ACT is free once the exp stream ends; split the copy
                nc.scalar.activation(ohat[:, 0:512], poh[:, 0:512], COPY)
                nc.vector.tensor_copy(ohat[:, 512:HW], poh[:, 512:HW])

            # finalize this half's 8 i-blocks (2 output groups of 4);
            # h=0's work overlaps h=1's exp stream.
            for g in range(2 * h, 2 * h + 2):
                stage = outp.tile([P, 4, D], f32, tag="stage", name="stage")
                for t in range(4):
                    i = 4 * g + t
                    po = psp.tile([P, EV], f32, tag="pp", name="po")
                    nc.tensor.matmul(
                        po, ohat[:, ts(i % 8, P)], rmat_s, start=True, stop=True
                    )
                    rec = outp.tile([P, 1], f32, tag="rec", name="rec")
                    nc.vector.reciprocal(rec, po[:, D:D + 1])
                    if h == 0:
                        nc.vector.tensor_scalar_mul(stage[:, t, :], po[:, 0:D], rec)
                    else:
                        nc.scalar.activation(
                            stage[:, t, :], po[:, 0:D], COPY, scale=rec
                        )
                nc.sync.dma_start(
                    out=out[g * 512:(g + 1) * 512, :].rearrange(
                        "(t p) e -> p t e", p=P
                    ),
                    in_=stage,
                )

    return nc


def dedup_ldweights(nc):
    """Drop Ldweights instructions that reload the exact weights already in
    the PE array (same AP, nothing clobbering in between).  The PE keeps the
    stationary operand across matmuls, so a back-to-back identical reload is
    pure dispatch overhead (~107ns each).  Only sync-free Ldweights are
    dropped so semaphore ordering is untouched."""
    dropped = 0
    for f in nc.m.functions:
        for blk in f.blocks:
            insts = list(blk.instructions)
            kept = []
            last_key = None
            for ins in insts:
                tname = type(ins).__name__
                if "PE" in str(getattr(ins, "engine", "")):
                    if tname == "InstLdweights":
                        ap = ins.ins[0]
                        key = (
                            ap.memref,
                            ap.offset,
                            str(ap.ap),
                            str(ap.dtype),
                            str(getattr(ins, "is_transpose", None)),
                        )
                        si = ins.sync_info
                        no_sync = si is None or (
                            len(si.on_wait) == 0 and len(si.on_update) == 0
                        )
                        if key == last_key and no_sync:
                            dropped += 1
                            continue
                        last_key = key
                    elif tname not in (
                        "InstMatmult",
                        "InstEventSemaphore",
                        "InstNoOp",
                        "InstDrain",
                    ):
                        last_key = None
                kept.append(ins)
            if len(kept) != len(insts):
                blk.instructions = kept
    return dropped


def prep_host(query, key, value, Wq, bq, Wk, bk, Wv, bv):
    """Host-side layout/algebra prep. Returns per-core input maps."""
    s = np.sqrt(np.float64(D))
    Wqp = np.concatenate([Wq, bq[:, None]], axis=1)  # [131, 132]
    Wkp = np.concatenate([Wk, bk[:, None]], axis=1)
    G = (Wqp.astype(np.float64).T @ Wkp.astype(np.float64)) / s  # [132, 132]
    U, S, Vt = np.linalg.svd(G)
    Aq = (U[:, :R] * np.sqrt(S[:R])).astype(np.float32)  # [132, 128]
    Ak = (Vt[:R, :].T * np.sqrt(S[:R])).astype(np.float32)

    W2 = np.zeros((DP, EV), np.float64)  # maps X -> [V | 1]
    W2[:D, :D] = Wv.T
    W2[D, :D] = bv
    W2[D, D] = 1.0
    U2, S2, V2t = np.linalg.svd(W2)
    L = (U2[:, :R] * np.sqrt(S2[:R])).astype(np.float32)  # [132, 128]
    Rm = (V2t[:R, :].T * np.sqrt(S2[:R])).astype(np.float32)  # [132, 128]

    wpack = np.concatenate([Aq, Ak, L], axis=1)  # [132, 384]
    wpack16 = np.ascontiguousarray(wpack.astype(_BF16))
    rmat16 = np.ascontiguousarray(Rm.T.astype(_BF16))  # [128, 132]

    ones_row = np.ones((1, N), np.float32)
    in_maps = []
    for c in range(NCORES):
        xs = [np.concatenate([x.T, ones_row], axis=0)
              for x in (query[c], key[c], value[c])]
        xallc = np.concatenate(xs, axis=1)  # [132, 6144]
        in_maps.append({
            "xall": np.ascontiguousarray(xallc.astype(_BF16)),
            "wpack": wpack16,
            "rmat": rmat16,
        })
    return in_maps


_NC_CACHE = {}


def _get_nc():
    if "nc" not in _NC_CACHE:
        nc = build_nc()
        if not nc.is_finalized():
            nc.finalize()  # Bacc.finalize runs the wait-split/EVSEM passes
        dedup_ldweights(nc)
        _NC_CACHE["nc"] = nc
    return _NC_CACHE["nc"]


def run_on_cores(in_maps, trace=False, **kw):
    from concourse.bass_utils import run_bass_kernel_spmd

    nc = _get_nc()
    return run_bass_kernel_spmd(nc, in_maps, core_ids=list(range(NCORES)),
                                trace=trace, **kw)


def kernel(query, key, value, Wq, bq, Wk, bk, Wv, bv):
    in_maps = prep_host(query, key, value, Wq, bq, Wk, bk, Wv, bv)
    res = run_on_cores(in_maps)
    return np.stack([np.asarray(res.results[c]["out"]) for c in range(NCORES)])


# revision 15
# speedup vs baseline: 1.0241x; 1.0241x over previous
"""Trainium2 Bass kernel for batched self-attention with input projections.

Problem: B=8, N=2048, D=131
    Q = q @ Wq.T + bq;  K = k @ Wk.T + bk;  V = v @ Wv.T + bv
    out = softmax(Q K^T / sqrt(131)) V

One batch element per NeuronCore (8 cores, no communication).

Host prep (layout/algebra only):
  - Tokens augmented with a ones-row: X = [x^T; 1] in [132, 2048] so biases
    fold into the projection matmuls.
  - Scores: Q K^T = Xq (Wq'^T Wk'/sqrt(D)) Xk^T = Xq G Xk^T, G [132,132].
    SVD-truncate G to rank 128 (exact rank 131; error ~2e-5) so the big S
    matmul is a single K=128 contraction:  S = (Xq Aq)(Xk Ak)^T.
  - Value path: W2 [132,132] maps X -> [V | 1] (bias row + denominator
    ones-column).  SVD-truncate W2 = L R^T to rank 128 so the O-matmul
    contracts into a 128-wide latent:  O' = (P Xv L) R^T, with O'[:,131]
    the softmax denominator.  Measured end-to-end rel err ~3.8e-3 in bf16.
  - Everything bf16 (PE runs bf16 at 4x fp32); fp32 PSUM accumulation.
    |S| < 3 so softmax without max-subtraction is safe.

Per core:
  QT[e',n] = Aq^T Xq, KT[e',n] = Ak^T Xk      (2 d-chunks: 0:128, 128:132)
  VL[n,l]  = Xv^T L                            (16 j-blocks of [128,128])
  for i-half h (1024 cols), j-block (16):
      ST = KT_j^T QT[:,h]  -> exp on ACT -> E (bf16, [128,1024])
      Ohat^T[l, h] += VL_j^T-matmul with E     (PSUM accumulate over j)
  O'[i,132] = Ohat_i R^T;  out = O'[:,0:131] / O'[:,131]

All SBUF projection tensors are chunked [128,512] tiles so the attention
stream starts as soon as its first chunks are projected; a burst of junk
matmuls during the input DMA warms the PE clock gate (HAM); a post-finalize
pass drops Ldweights instructions that reload the identical weights.
"""

import numpy as np
import ml_dtypes

P = 128          # partitions / PE width
N = 2048         # tokens per core
D = 131          # embed dim
DP = 132         # embed dim + ones row
DLO = DP - P     # tail contraction rows (4)
R = 128          # truncated rank (QK interaction and V latent)
EV = 132         # final output cols (131 + denominator)
NB = N // P      # 16 token blocks
HW = 1024        # i-half width
NH = N // HW     # 2 halves
NCORES = 8

QOFF, KOFF, VOFF = 0, N, 2 * N          # column offsets in packed xall
AQOFF, AKOFF, LOFF = 0, R, 2 * R        # column offsets in packed weights

_BF16 = ml_dtypes.bfloat16


def build_nc():
    """Build the single-core Bass graph (same NEFF runs SPMD on all 8 cores)."""
    from contextlib import ExitStack

    import concourse.bacc as bacc
    import concourse.mybir as mybir
    import concourse.tile as tile
    from concourse.bass import ts

    bf = mybir.dt.bfloat16
    f32 = mybir.dt.float32
    EXP = mybir.ActivationFunctionType.Exp
    COPY = mybir.ActivationFunctionType.Copy

    nc = bacc.Bacc()
    xall = nc.declare_dram_parameter("xall", [DP, 3 * N], bf, isOutput=False)
    wpack = nc.declare_dram_parameter("wpack", [DP, 3 * R], bf, isOutput=False)
    rmat = nc.declare_dram_parameter("rmat", [R, EV], bf, isOutput=False)
    out = nc.declare_dram_parameter("out", [N, D], f32, isOutput=True)

    with tile.TileContext(nc) as tc, ExitStack() as ctx:
        const = ctx.enter_context(tc.tile_pool(name="const", bufs=1))
        xin = ctx.enter_context(tc.tile_pool(name="xin", bufs=1))
        proj = ctx.enter_context(tc.tile_pool(name="proj", bufs=1))
        epool = ctx.enter_context(tc.tile_pool(name="epool", bufs=NB + 2))
        ohs = ctx.enter_context(tc.tile_pool(name="ohs", bufs=1))
        outp = ctx.enter_context(tc.tile_pool(name="outp", bufs=2))
        warm = ctx.enter_context(tc.tile_pool(name="warm", bufs=1))
        # PSUM budget (8 banks): proj/final 2 x [128,512] = 2, scores
        # 2 x [128,1024] = 4, Ohat accumulator 1 x [128,1024] = 2.
        psp = ctx.enter_context(tc.tile_pool(name="psp", bufs=2, space="PSUM"))
        psst = ctx.enter_context(tc.tile_pool(name="psst", bufs=2, space="PSUM"))
        psoh = ctx.enter_context(tc.tile_pool(name="psoh", bufs=1, space="PSUM"))

        # ---- DMA loads.  The big inputs are split into 4 partition-range
        # chunks each: 32 descriptors per dma_start (fast issue) and the
        # chunks spread over the 8 HWDGE queues so transfers run parallel.
        wp_hi = const.tile([P, 3 * R], bf)
        nc.sync.dma_start(out=wp_hi, in_=wpack[0:P, :])
        wp_lo = const.tile([DLO, 3 * R], bf)
        nc.sync.dma_start(out=wp_lo, in_=wpack[P:DP, :])
        xall_hi = xin.tile([P, 3 * N], bf)
        xall_lo = xin.tile([DLO, 3 * N], bf)
        nc.sync.dma_start(out=xall_lo, in_=xall[P:DP, :])
        for off in (QOFF, KOFF, VOFF):
            for s in range(0, P, 32):
                nc.sync.dma_start(
                    out=xall_hi[s:s + 32, off:off + N],
                    in_=xall[s:s + 32, off:off + N],
                )
        rmat_s = const.tile([R, EV], bf)
        nc.sync.dma_start(out=rmat_s, in_=rmat[:, :])

        # ---- PE warm-up during the DMA window: dense junk matmuls flip the
        # HAM clock gate (~3.4us of sustained activity) so the projections
        # run at 2.4GHz.  No data deps -> the scheduler front-loads them.
        wsrc = warm.tile([P, 512], bf)
        nc.vector.memset(wsrc, 0)
        for w in range(26):
            pw = psst.tile([P, HW], f32, tag="pst", name="pw")
            nc.tensor.matmul(pw[:, 0:512], wsrc[:, 0:P], wsrc, start=True, stop=True)

        # ---- projections, chunked so attention can start early.
        # Order: the h=0 attention stream needs QT chunks 0-1 and all KT
        # chunks, so project those first; QT 2-3 (h=1) and VL (Ohat) follow.
        qts = [proj.tile([P, 512], bf, tag=f"qt{c}", name=f"qt{c}") for c in range(4)]
        kts = [proj.tile([P, 512], bf, tag=f"kt{c}", name=f"kt{c}") for c in range(4)]
        vls = [proj.tile([P, 512], bf, tag=f"vl{c}", name=f"vl{c}") for c in range(4)]

        def qk_chunk(dst, woff, xoff, c):
            pp = psp.tile([P, 512], f32, tag="pp", name="pp")
            nc.tensor.matmul(
                pp,
                wp_hi[:, woff:woff + R],
                xall_hi[:, xoff + c * 512: xoff + (c + 1) * 512],
                start=True,
                stop=False,
            )
            nc.tensor.matmul(
                pp,
                wp_lo[:, woff:woff + R],
                xall_lo[:, xoff + c * 512: xoff + (c + 1) * 512],
                start=False,
                stop=True,
            )
            nc.vector.tensor_copy(dst, pp)

        for c in range(2):
            qk_chunk(qts[c], AQOFF, QOFF, c)
        for c in range(4):
            qk_chunk(kts[c], AKOFF, KOFF, c)
        for c in range(2, 4):
            qk_chunk(qts[c], AQOFF, QOFF, c)
        for g in range(4):
            pv = psp.tile([P, 512], f32, tag="pp", name="pv")
            for t in range(4):
                j = 4 * g + t
                nc.tensor.matmul(
                    pv[:, ts(t, P)],
                    xall_hi[:, VOFF + j * P: VOFF + (j + 1) * P],
                    wp_hi[:, LOFF:LOFF + R],
                    start=True,
                    stop=False,
                )
                nc.tensor.matmul(
                    pv[:, ts(t, P)],
                    xall_lo[:, VOFF + j * P: VOFF + (j + 1) * P],
                    wp_lo[:, LOFF:LOFF + R],
                    start=False,
                    stop=True,
                )
            nc.vector.tensor_copy(vls[g], pv)

        # ---- attention + per-half finalization ----
        for h in range(NH):
            es = []
            for j in range(NB):
                pst = psst.tile([P, HW], f32, tag="pst", name="pst")
                for c in range(2):
                    nc.tensor.matmul(
                        pst[:, ts(c, 512)],
                        kts[j // 4][:, ts(j % 4, P)],
                        qts[2 * h + c],
                        start=True,
                        stop=True,
                    )
                ej = epool.tile([P, HW], bf, tag="e", name="ej")
                nc.scalar.activation(ej, pst, EXP)
                es.append(ej)
            poh = psoh.tile([P, HW], f32, tag="poh", name="poh")
            for j in range(NB):
                for c in range(2):
                    nc.tensor.matmul(
                        poh[:, ts(c, 512)],
                        vls[j // 4][:, ts(j % 4, P)],
                        es[j][:, ts(c, 512)],
                        start=(j == 0),
                        stop=(j == NB - 1),
                    )
            ohat = ohs.tile([P, HW], bf, tag=f"oh{h}", name=f"oh{h}")
            if h == 0:
                nc.vector.tensor_copy(ohat, poh)
            else:
                # ACT is free once the exp stream ends; split the copy
                nc.scalar.activation(ohat[:, 0:512], poh[:, 0:512], COPY)
                nc.vector.tensor_copy(ohat[:, 512:HW], poh[:, 512:HW])

            # finalize this half's 8 i-blocks (2 output groups of 4);
            # h=0's work overlaps h=1's exp stream.
            for g in range(2 * h, 2 * h + 2):
                stage = outp.tile([P, 4, D], f32, tag="stage", name="stage")
                for t in range(4):
                    i = 4 * g + t
                    po = psp.tile([P, EV], f32, tag="pp", name="po")
                    nc.tensor.matmul(
                        po, ohat[:, ts(i % 8, P)], rmat_s, start=True, stop=True
                    )
                    rec = outp.tile([P, 1], f32, tag="rec", name="rec")
                    nc.vector.reciprocal(rec, po[:, D:D + 1])
                    if h == 0:
                        nc.vector.tensor_scalar_mul(stage[:, t, :], po[:, 0:D], rec)
                    else:
                        nc.scalar.activation(
                            stage[:, t, :], po[:, 0:D], COPY, scale=rec
                        )
                nc.sync.dma_start(
                    out=out[g * 512:(g + 1) * 512, :].rearrange(
                        "(t p) e -> p t e", p=P
                    ),
                    in_=stage,
                )

    return nc


def dedup_ldweights(nc):
    """Drop Ldweights instructions that reload the exact weights already in
    the PE array (same AP, nothing clobbering in between).  The PE keeps the
    stationary operand across matmuls, so a back-to-back identical reload is
    pure dispatch overhead (~107ns each).  Only sync-free Ldweights are
    dropped so semaphore ordering is untouched."""
    dropped = 0
    for f in nc.m.functions:
        for blk in f.blocks:
            insts = list(blk.instructions)
            kept = []
            last_key = None
            for ins in insts:
                tname = type(ins).__name__
                if "PE" in str(getattr(ins, "engine", "")):
                    if tname == "InstLdweights":
                        ap = ins.ins[0]
                        key = (
                            ap.memref,
                            ap.offset,
                            str(ap.ap),
                            str(ap.dtype),
                            str(getattr(ins, "is_transpose", None)),
                        )
                        si = ins.sync_info
                        no_sync = si is None or (
                            len(si.on_wait) == 0 and len(si.on_update) == 0
                        )
                        if key == last_key and no_sync:
                            dropped += 1
                            continue
                        last_key = key
                    elif tname not in (
                        "InstMatmult",
                        "InstEventSemaphore",
                        "InstNoOp",
                        "InstDrain",
                    ):
                        last_key = None
                kept.append(ins)
            if len(kept) != len(insts):
                blk.instructions = kept
    return dropped


def prep_host(query, key, value, Wq, bq, Wk, bk, Wv, bv):
    """Host-side layout/algebra prep. Returns per-core input maps."""
    s = np.sqrt(np.float64(D))
    Wqp = np.concatenate([Wq, bq[:, None]], axis=1)  # [131, 132]
    Wkp = np.concatenate([Wk, bk[:, None]], axis=1)
    G = (Wqp.astype(np.float64).T @ Wkp.astype(np.float64)) / s  # [132, 132]
    U, S, Vt = np.linalg.svd(G)
    Aq = (U[:, :R] * np.sqrt(S[:R])).astype(np.float32)  # [132, 128]
    Ak = (Vt[:R, :].T * np.sqrt(S[:R])).astype(np.float32)

    W2 = np.zeros((DP, EV), np.float64)  # maps X -> [V | 1]
    W2[:D, :D] = Wv.T
    W2[D, :D] = bv
    W2[D, D] = 1.0
    U2, S2, V2t = np.linalg.svd(W2)
    L = (U2[:, :R] * np.sqrt(S2[:R])).astype(np.float32)  # [132, 128]
    Rm = (V2t[:R, :].T * np.sqrt(S2[:R])).astype(np.float32)  # [132, 128]

    wpack = np.concatenate([Aq, Ak, L], axis=1)  # [132, 384]
    wpack16 = np.ascontiguousarray(wpack.astype(_BF16))
    rmat16 = np.ascontiguousarray(Rm.T.astype(_BF16))  # [128, 132]

    ones_row = np.ones((1, N), np.float32)
    in_maps = []
    for c in range(NCORES):
        xs = [np.concatenate([x.T, ones_row], axis=0)
              for x in (query[c], key[c], value[c])]
        xallc = np.concatenate(xs, axis=1)  # [132, 6144]
        in_maps.append({
            "xall": np.ascontiguousarray(xallc.astype(_BF16)),
            "wpack": wpack16,
            "rmat": rmat16,
        })
    return in_maps


_NC_CACHE = {}


def _get_nc():
    if "nc" not in _NC_CACHE:
        nc = build_nc()
        if not nc.is_finalized():
            nc.finalize()  # Bacc.finalize runs the wait-split/EVSEM passes
        dedup_ldweights(nc)
        _NC_CACHE["nc"] = nc
    return _NC_CACHE["nc"]


def run_on_cores(in_maps, trace=False, **kw):
    from concourse.bass_utils import run_bass_kernel_spmd

    nc = _get_nc()
    return run_bass_kernel_spmd(nc, in_maps, core_ids=list(range(NCORES)),
                                trace=trace, **kw)


def kernel(query, key, value, Wq, bq, Wk, bk, Wv, bv):
    in_maps = prep_host(query, key, value, Wq, bq, Wk, bk, Wv, bv)
    res = run_on_cores(in_maps)
    return np.stack([np.asarray(res.results[c]["out"]) for c in range(NCORES)])
